# revision 2
# baseline (speedup 1.0000x reference)
"""Horizontal correlation cost volume on 8 Trainium2 NeuronCores.

out[b, ctr, h, w] = sum_c a[b, c, h, w] * b_[b, c, h, w - (D - ctr)],  D = 40.

Sharding: data-parallel over batch B=8, one batch element per core.

Per-core device algorithm (a_i, b_i: [C=128, H=192, W=256] -> bf16):
  For each h row and each 128-wide w tile, 4 column-tiled bf16 matmuls
  (tile_position col groups g) compute a compact displacement band
    psum[32g + m0, j] = sum_c a[c, w0 + 32g + m0] * b[c, w0 + 32g + j - 40]
  for j in [0,72); the 41 displacement values for output column w = w0+32g+m0
  sit at j = m0..m0+40 of partition 32g+m0.  Out-of-image b columns (only the
  first w-tile's groups g=0,1) are skipped by clipping the moving operand; the
  affected psum region is garbage and the host zeroes the corresponding
  (w + ctr < 40) output triangle, which is exactly zero by definition.

  Inputs are converted to bf16 on the host (halves input DMA, 4x matmul
  throughput vs fp32; accumulation stays fp32 in PSUM, total rel err ~1e-3
  against the 2e-2 gate).  Band tiles are down-converted to bf16 by the
  PSUM->SBUF copy and staged rectangularly to DRAM; the host performs the
  final diagonal re-indexing (a pure layout gather of device-computed
  values — per-partition byte offsets are unsupported by the DMA AP
  lowering, so the diagonal cannot be extracted on-device).
"""
import sys

if "/opt/trn_rl_repo" not in sys.path:
    sys.path.insert(0, "/opt/trn_rl_repo")

import numpy as np

C, H, W, D = 128, 192, 256, 40
DCT = D + 1          # 41 displacements
T = 128              # w-tile width (psum partitions)
R = 16               # h rows per strip
G = 4                # col-tile groups per w-tile
GW = T // G          # 32 output columns per group
NJ = GW + D          # 72 band columns per group
NSTRIP = H // R      # 12
WT = W // T          # 2
NBUF = 4             # strip pipeline depth

_CACHE = {}


def build(for_i_n=None):
    """Build the per-core Bass module.  for_i_n: wrap the strip loop in a
    hardware For_i repeating the whole body n times (timing harness)."""
    import concourse.bacc as bacc
    import concourse.mybir as mybir
    import concourse.tile as tile
    from contextlib import ExitStack

    f32 = mybir.dt.float32
    bf16 = mybir.dt.bfloat16
    nc = bacc.Bacc("TRN2", target_bir_lowering=False, debug=False, num_devices=8)
    a_d = nc.dram_tensor("a", [C, H, W], bf16, kind="ExternalInput")
    b_d = nc.dram_tensor("b", [C, H, W], bf16, kind="ExternalInput")
    st_d = nc.dram_tensor("st", [T, NSTRIP, WT * R, NJ], bf16,
                          kind="ExternalOutput")

    with tile.TileContext(nc) as tc:
        with (
            tc.tile_pool(name="persist", bufs=1) as pp,
            tc.tile_pool(name="ps", bufs=8, space="PSUM") as psp,
        ):
            A_sb = [pp.tile([C, R, W], bf16, tag=f"a{k}", name=f"a{k}")
                    for k in range(NBUF)]
            B_sb = [pp.tile([C, R, W], bf16, tag=f"b{k}", name=f"b{k}")
                    for k in range(NBUF)]
            S_sb = [pp.tile([T, WT * R, NJ], bf16, tag=f"s{k}", name=f"s{k}")
                    for k in range(NBUF)]

            def body():
                for s in range(NSTRIP):
                    k = s % NBUF
                    h0 = s * R
                    hh = R // 2
                    nc.sync.dma_start(A_sb[k][:, 0:hh, :],
                                      a_d.ap()[:, h0:h0 + hh, :])
                    nc.scalar.dma_start(B_sb[k][:, 0:hh, :],
                                        b_d.ap()[:, h0:h0 + hh, :])
                    nc.sync.dma_start(A_sb[k][:, hh:R, :],
                                      a_d.ap()[:, h0 + hh:h0 + R, :])
                    nc.scalar.dma_start(B_sb[k][:, hh:R, :],
                                        b_d.ap()[:, h0 + hh:h0 + R, :])
                    for wt in range(WT):
                        for h in range(R):
                            psum = psp.tile([T, NJ], f32)
                            for g in range(G):
                                bcol0 = wt * T + GW * g - D
                                clip = max(0, -bcol0)
                                nc.tensor.matmul(
                                    psum[GW * g:GW * (g + 1), clip:NJ],
                                    A_sb[k][:, h,
                                            wt * T + GW * g:wt * T + GW * (g + 1)],
                                    B_sb[k][:, h, bcol0 + clip:bcol0 + NJ],
                                    start=True, stop=True,
                                    tile_position=(0, GW * g),
                                )
                            nc.vector.tensor_copy(
                                S_sb[k][:, wt * R + h, :], psum[:])
                    nc.gpsimd.dma_start(st_d.ap()[:, s, :, :], S_sb[k][:, :, :])

            if for_i_n is None:
                body()
            else:
                with tc.For_i(0, for_i_n):
                    body()

    nc.compile()
    return nc


def _get_nc():
    if "nc" not in _CACHE:
        _CACHE["nc"] = build()
    return _CACHE["nc"]


def _assemble(results):
    """Host-side diagonal extraction from the staged band tiles."""
    # per-core st: [T=128, NSTRIP, WT*R, NJ] bf16
    st = np.stack([np.asarray(results[i]["st"], dtype=np.float32)
                   for i in range(8)])
    # partition p = 32g + m0; reorder to [8, WT, NSTRIP, G, GW, R, NJ]
    st = st.reshape(8, G, GW, NSTRIP, WT, R, NJ).transpose(0, 4, 3, 1, 2, 5, 6)
    m0 = np.arange(GW)
    out = np.empty((8, DCT, NSTRIP, R, WT, G, GW), np.float32)
    for ctr in range(DCT):
        # advanced indexing over (m0-axis4, j-axis6) -> [GW, 8, WT, NSTRIP, G, R]
        dg = st[:, :, :, :, m0, :, m0 + ctr]
        out[:, ctr] = dg.transpose(1, 3, 5, 2, 4, 0)
    out = out.reshape(8, DCT, H, W)
    # zero the w + ctr < 40 triangle (b column out of image)
    wg = np.arange(W)[None, :]
    cg = np.arange(DCT)[:, None]
    mask = (wg + cg) < D                      # [DCT, W]
    return np.where(mask[None, :, None, :], np.float32(0.0), out)


def run(a, b, trace=False):
    """a, b: [8, C, H, W] fp32. Returns (out [8, DCT, H, W], BassKernelResults)."""
    import ml_dtypes
    from concourse import bass_utils

    nc = _get_nc()
    a = np.ascontiguousarray(np.asarray(a).astype(ml_dtypes.bfloat16))
    b = np.ascontiguousarray(np.asarray(b).astype(ml_dtypes.bfloat16))
    in_maps = [{"a": a[i], "b": b[i]} for i in range(8)]
    res = bass_utils.run_bass_kernel_spmd(
        nc, in_maps, core_ids=list(range(8)), trace=trace
    )
    out = _assemble(res.results)
    return out, res


def kernel(a, b, max_displacement):
    assert int(max_displacement) == D
    out, _ = run(a, b)
    return out
